# revision 27
# baseline (speedup 1.0000x reference)
"""LMU (Legendre Memory Unit) RNN kernel for Trainium2, 8 NeuronCores.

Strategy
--------
Data-parallel over batch: each of the 8 cores runs B_local = 16 sequences
through the full T=784-step recurrence; outputs are concatenated on host.

Per-step math is algebraically folded into a single affine map (same fold
as before).  With u_t = e_x x_t + h_t e_h^T + m_t e_m^T, Ad = I + AT,
w = W_m @ BT:

    m_{t+1} = (Ad + BT e_m) m_t + (BT e_h) h_t + (BT e_x) x_t
    pre_h   = (W_h + w e_h) h_t + (W_m Ad + w e_m) m_t + (W_x + w e_x) x_t
    h_{t+1} = tanh(pre_h)

so each step is one matmul  out[16, 1536] = s[16, 1537] @ M_big.T  plus a
tanh on the h columns.  M_big is precomputed on host in float64, fp16.

On-chip schedule (v2: PE column tiling).  The per-core batch is only 16,
so each matmul's stationary operand (a state K-tile, [128, 16]) occupies
just 16 of the PE array's 128 columns.  The 1536 output features are
split into 4 strips of 384, and the 4 strips' matmuls are issued to the
four 32-column groups of the PE array (tile_position=(0, 32j), derived
automatically from the PSUM slice base partition).  The 4 weight streams
then run concurrently, cutting the per-step matmul span ~4x.

The four PSUM strips live at partitions {0-15, 32-47, 64-79, 96-111} of
one [128, 384] PSUM tile, so the whole step output is re-transposed with
just 3 full-width (128,128) PE transposes (instead of 12 thin ones).
Output features are permuted on host so that strip columns 0:256 are h
(tanh) and 256:384 are m (copy); after transposing, chunk 0/1 are pure h
and chunk 2 is pure m, letting tanh run as two [128,128] ACT ops at full
lane utilization, off the critical path of the next step's first rounds.

Feature order bookkeeping (host-side permutations):
  out-slot (j, c):   OP[j*384+c] = 256j+c        (c < 256, h)
                                  = 1024+128j+c-256 (c >= 256, m)
  state tile (t, j): IP[(t*4+j)*128+c] = 256j+128t+c (t<2, h)
                                       = 1024+128j+c (t=2, m)
The state tile (t, j) is st[:, 128t+32j : 128t+32j+16] after transpose,
and weight block r = t*4+j rows follow IP so state/weights stay aligned.
"""

import numpy as np

import concourse.bass as bass
import concourse.mybir as mybir
import concourse.tile as tile
from concourse import bacc
from concourse.bass import ds, ts
from concourse.bass_utils import run_bass_kernel_spmd
from concourse.masks import make_identity

T, B, UNITS, ORDER = 784, 128, 1024, 512
NCORES = 8
BL = B // NCORES          # 16 sequences per core
SOUT = UNITS + ORDER      # 1536 output features [pre_h; m']
KT = 12                   # state K-tiles of 128
NSTRIP = 4                # PE column groups
NW = SOUT // NSTRIP       # 384 output columns per strip

FP16 = mybir.dt.float16
FP32 = mybir.dt.float32


def _perms():
    OP = np.empty(SOUT, np.int64)
    for j in range(NSTRIP):
        for c in range(NW):
            OP[j * NW + c] = 256 * j + c if c < 256 else 1024 + 128 * j + (c - 256)
    IP = np.empty(SOUT, np.int64)
    for t in range(3):
        for j in range(4):
            for c in range(128):
                IP[(t * 4 + j) * 128 + c] = (
                    256 * j + 128 * t + c if t < 2 else 1024 + 128 * j + c
                )
    return OP, IP


def _build_weights(e_x, e_h, e_m, W_x, W_h, W_m, AT, BT):
    """Host-side fold into M_big (float64), then permute rows/cols."""
    f = np.float64
    e_x, e_h, e_m = e_x.astype(f), e_h.astype(f), e_m.astype(f)
    W_x, W_h, W_m = W_x.astype(f), W_h.astype(f), W_m.astype(f)
    AT, BT = AT.astype(f), BT.astype(f)
    Ad = np.eye(ORDER) + AT
    w = W_m @ BT                                   # (U, 1)
    top = np.concatenate([W_h + w @ e_h, W_m @ Ad + w @ e_m, W_x + w * e_x], axis=1)
    bot = np.concatenate([BT @ e_h, Ad + BT @ e_m, BT * e_x], axis=1)
    M_big = np.concatenate([top, bot], axis=0)     # (1536 out, 1537 in)
    OP, IP = _perms()
    Wstate = M_big[np.ix_(OP, IP)].T               # (in-perm, out-slot)
    xw = M_big[OP, SOUT]                           # x column, out-slot order
    return Wstate.astype(np.float16), xw.reshape(1, SOUT).astype(np.float16)


def _build_nc(t_steps=T, unroll=56):
    assert t_steps % unroll == 0 and unroll % 2 == 0
    iters = t_steps // unroll
    nc = bacc.Bacc("TRN2", target_bir_lowering=False, num_devices=NCORES)

    w_dram = nc.dram_tensor("wts", [KT * 128, SOUT], FP16, kind="ExternalInput")
    xw_dram = nc.dram_tensor("xw", [NSTRIP, NW], FP16, kind="ExternalInput")
    isel_dram = nc.dram_tensor("isel", [128, 64], FP16, kind="ExternalInput")
    wd_dram = nc.dram_tensor("wd", [UNITS + 1, 10], FP16, kind="ExternalInput")
    x_dram = nc.dram_tensor("xs", [NSTRIP, t_steps * 128], FP16, kind="ExternalInput")
    out_dram = nc.dram_tensor("out", [BL, 10], FP32, kind="ExternalOutput")

    TANH = mybir.ActivationFunctionType.Tanh

    with tile.TileContext(nc) as tc:
        with (
            tc.tile_pool(name="const", bufs=1) as cpool,
            tc.tile_pool(name="state", bufs=1) as spool,
            tc.tile_pool(name="work", bufs=2) as wpool,
            tc.tile_pool(name="psum", bufs=1, space="PSUM") as ppool,
        ):
            # ---- persistent SBUF ----
            w_sb = cpool.tile([128, KT * SOUT], FP16, tag="w_sb")
            for r in range(KT):
                nc.sync.dma_start(w_sb[:, ts(r, SOUT)], w_dram[ts(r, 128), :])
            xw_sb = cpool.tile([NSTRIP, NW], FP16, tag="xw_sb")
            nc.sync.dma_start(xw_sb[:, :], xw_dram[:, :])
            wd_sb = cpool.tile([128, 8 * 10], FP16, tag="wd_sb")
            for r in range(8):
                nc.sync.dma_start(wd_sb[:, ts(r, 10)], wd_dram[ts(r, 128), :])
            bias_sb = cpool.tile([1, 10], FP16, tag="bias_sb")
            nc.sync.dma_start(bias_sb[:, :], wd_dram[1024:1025, :])
            ones_sb = cpool.tile([1, BL], FP16, tag="ones_sb")
            nc.vector.memset(ones_sb[:, :], 1.0)
            # transpose selector: isel[p, 16j+b] = (p == 32j+b), so
            # sf_chunk.T @ isel extracts only the 64 valid (strip, batch)
            # columns of the transposed chunk in packed form
            isel_sb = cpool.tile([128, 64], FP16, tag="isel_sb")
            nc.sync.dma_start(isel_sb[:, :], isel_dram[:, :])

            stA = spool.tile([128, 3 * 64], FP16, tag="stA")
            stB = spool.tile([128, 3 * 64], FP16, tag="stB")
            sfA = spool.tile([128, NW], FP16, tag="sfA")
            sfB = spool.tile([128, NW], FP16, tag="sfB")
            for t_ in (stA, stB, sfA, sfB):
                nc.vector.memset(t_[:, :], 0.0)
            x_stage = spool.tile([NSTRIP, unroll * 128], FP16, tag="x_stage")

            psA = ppool.tile([128, NW], FP32, tag="psA")
            psB = ppool.tile([128, NW], FP32, tag="psB")
            tpA = ppool.tile([128, 3 * 64], FP32, tag="tpA")
            tpB = ppool.tile([128, 3 * 64], FP32, tag="tpB")

            def xround(ps, x_ap):
                # all four strips' rank-1 x contribution in ONE K=4 matmul:
                # lhsT row k holds x_t at columns 32k:32k+16 (host-built
                # spread), rhs row k holds xw for strip k.  Writes the full
                # [128, NW] tile (start=True), which also zeroes the
                # partitions the strip matmuls never touch.
                nc.tensor.matmul(
                    ps[:, :], x_ap, xw_sb[:, :], start=True, stop=False,
                )

            def step(src, dst, ps, ps_next, tp, sf, x_next):
                # next step's x round first: its deps (x_stage, ps_next) are
                # ready early, so the scheduler can slot it into the PE gap
                # while this step waits on the previous tanh
                if x_next is not None:
                    xround(ps_next, x_next)
                # 12 K-rounds; within a round the 4 strip matmuls go to the
                # four PE column groups and stream concurrently.  (PSUM
                # dependency tracking is tile-granular, so column-splitting
                # rounds to unblock CAST1 early does NOT work — the reader
                # waits for every writer of the tile regardless.)
                for r in range(KT):
                    lhsT = src[:, ds(64 * (r // 4) + 16 * (r % 4), BL)]
                    for j in range(NSTRIP):
                        nc.tensor.matmul(
                            ps[32 * j : 32 * j + BL, :],
                            lhsT,
                            w_sb[:, ds(r * SOUT + j * NW, NW)],
                            start=False,
                            stop=(r == KT - 1),
                            tile_position=(0, 32 * j),
                        )
                # raw psum -> sbuf fp16 (pre-activation); chunk 0 first so
                # the critical chain to the next step's first rounds
                # unblocks asap
                nc.vector.tensor_copy(sf[:, 0:128], ps[:, 0:128])
                nc.vector.tensor_copy(sf[:, 128:NW], ps[:, 128:NW])
                # packed transpose via selector matmul (N=64, fp32 out)
                nc.tensor.matmul(
                    tp[:, 0:64], sf[:, 0:128], isel_sb[:, :],
                    start=True, stop=True,
                )
                # finalize state: tanh on h chunks, copy on m chunk
                nc.scalar.activation(dst[:, 0:64], tp[:, 0:64], TANH)
                nc.tensor.matmul(
                    tp[:, 64:128], sf[:, 128:256], isel_sb[:, :],
                    start=True, stop=True,
                )
                nc.scalar.activation(dst[:, 64:128], tp[:, 64:128], TANH)
                nc.tensor.matmul(
                    tp[:, 128:192], sf[:, 256:384], isel_sb[:, :],
                    start=True, stop=True,
                )
                nc.vector.tensor_copy(dst[:, 128:192], tp[:, 128:192])

            with tc.For_i(0, iters, hint_engines=(mybir.EngineType.PE,)) as i:
                nc.sync.dma_start(
                    x_stage[:, :], x_dram[:, ds(i * (unroll * 128), unroll * 128)]
                )
                xround(psA, x_stage[:, 0:128])
                for u in range(unroll):
                    src, dst = (stA, stB) if u % 2 == 0 else (stB, stA)
                    ps, psn = (psA, psB) if u % 2 == 0 else (psB, psA)
                    tp = tpA if u % 2 == 0 else tpB
                    sf = sfA if u % 2 == 0 else sfB
                    x_next = (
                        x_stage[:, ts(u + 1, 128)] if u < unroll - 1 else None
                    )
                    step(src, dst, ps, psn, tp, sf, x_next)

            # ---- epilogue: logits = h W_d^T + b ; softmax ----
            # final state is in stA (t_steps even); h = chunks t=0,1
            ps_l = ppool.tile([BL, 10], FP32, tag="ps_l")
            for kt in range(8):
                nc.tensor.matmul(
                    ps_l[:, :],
                    stA[:, ds(64 * (kt // 4) + 16 * (kt % 4), BL)],
                    wd_sb[:, ts(kt, 10)],
                    start=(kt == 0),
                    stop=False,
                )
            nc.tensor.matmul(
                ps_l[:, :], ones_sb[:, :], bias_sb[:, :], start=False, stop=True
            )
            sm = wpool.tile([BL, 10], FP32, tag="sm")
            nc.scalar.activation(sm[:, :], ps_l[:, :], mybir.ActivationFunctionType.Exp)
            ssum = wpool.tile([BL, 1], FP32, tag="ssum")
            nc.vector.reduce_sum(ssum[:, :], sm[:, :], axis=mybir.AxisListType.X)
            srec = wpool.tile([BL, 1], FP32, tag="srec")
            nc.vector.reciprocal(srec[:, :], ssum[:, :])
            nc.vector.tensor_scalar_mul(sm[:, :], sm[:, :], srec[:, :])
            nc.sync.dma_start(out_dram[:, :], sm[:, :])

    nc.compile()
    return nc


_NC_CACHE = {}


def _get_nc(t_steps=T, unroll=56):
    key = (t_steps, unroll)
    if key not in _NC_CACHE:
        _NC_CACHE[key] = _build_nc(t_steps, unroll)
    return _NC_CACHE[key]


def kernel(inputs, e_x, e_h, e_m, W_x, W_h, W_m, AT, BT, W_dense, b_dense,
           _t_steps=T, _unroll=56, _trace=False):
    inputs = np.asarray(inputs, np.float32)
    args = [np.asarray(a, np.float32)
            for a in (e_x, e_h, e_m, W_x, W_h, W_m, AT, BT, W_dense, b_dense)]
    e_x, e_h, e_m, W_x, W_h, W_m, AT, BT, W_dense, b_dense = args

    wts, xw = _build_weights(e_x, e_h, e_m, W_x, W_h, W_m, AT, BT)
    xw = np.ascontiguousarray(xw.reshape(NSTRIP, NW))
    _, IP = _perms()
    wd = np.zeros((UNITS + 1, 10), np.float16)
    wd[:UNITS, :] = W_dense.T[IP[:UNITS], :].astype(np.float16)
    wd[UNITS, :] = b_dense.astype(np.float16)

    isel = np.zeros((128, 64), np.float16)
    for j in range(NSTRIP):
        for b in range(BL):
            isel[32 * j + b, 16 * j + b] = 1.0

    x = inputs[:_t_steps, :, 0].astype(np.float16)        # (T, B)
    nc = _get_nc(_t_steps, _unroll)
    in_maps = []
    for c in range(NCORES):
        xc = x[:, c * BL:(c + 1) * BL]                    # (T, BL)
        # x-spread for the single K=4 x matmul: row k carries x_t at
        # columns t*128 + 32k + b
        xs4 = np.zeros((NSTRIP, _t_steps, 128), np.float16)
        for k in range(NSTRIP):
            xs4[k, :, 32 * k : 32 * k + BL] = xc
        xs4 = np.ascontiguousarray(xs4.reshape(NSTRIP, _t_steps * 128))
        in_maps.append(
            {"wts": wts, "xw": xw, "wd": wd, "xs": xs4, "isel": isel}
        )

    res = run_bass_kernel_spmd(
        nc, in_maps, core_ids=list(range(NCORES)), trace=_trace
    )
    out = np.concatenate([res.results[c]["out"] for c in range(NCORES)], axis=0)
    kernel.last_results = res
    return out.astype(np.float32)


# revision 28
# speedup vs baseline: 1.2245x; 1.2245x over previous
"""LMU (Legendre Memory Unit) RNN kernel for Trainium2, 8 NeuronCores.

Strategy
--------
Data-parallel over batch: each of the 8 cores runs B_local = 16 sequences
through the full T=784-step recurrence; outputs are concatenated on host.

Per-step math is algebraically folded into a single affine map.  With
u_t = e_x x_t + h_t e_h^T + m_t e_m^T, Ad = I + AT, w = W_m @ BT:

    m_{t+1} = (Ad + BT e_m) m_t + (BT e_h) h_t + (BT e_x) x_t
    pre_h   = (W_h + w e_h) h_t + (W_m Ad + w e_m) m_t + (W_x + w e_x) x_t
    h_{t+1} = tanh(pre_h)

so each step is one matmul  out[16, 1536] = s[16, 1537] @ M_big.T  plus a
tanh on the h columns.  M_big is precomputed on host in float64 -> fp16.

On-chip schedule (PE column tiling).  The per-core batch is only 16, so
each matmul's stationary operand (a state K-tile, [128, 16]) occupies
just 16 of the PE array's 128 columns.  The 1536 output features are
split into 4 strips of 384 issued to the four 32-column groups of the PE
array (tile_position=(0, 32j)); the 4 weight streams run concurrently,
cutting the per-step matmul span ~4x (measured round pitch ~163ns =
384 cols @ 2.4 GHz, stream-bound).

The four PSUM strips live at partitions {0-15, 32-47, 64-79, 96-111} of
one [128, 384] PSUM tile, so the step output is re-transposed with just
3 full-width (128,128) PE transposes (instead of 12 thin ones).  Output
features are permuted on host so each strip is 256 h columns + 128 m
columns; after transposing, chunk 0/1 are pure h (tanh on ACT at full
lane utilization) and chunk 2 is pure m (DVE copy).

The per-step x contribution (rank-1 per strip) is ONE K=4 matmul: lhsT
row k holds x_t spread at columns 32k:32k+16 (host-built), rhs row k is
strip k's xw.  Its full-width start=True write also keeps the
never-again-touched psum partitions (16:32 of each 32-group) zeroed.

Feature order bookkeeping (host-side permutations):
  out-slot (j, c):   OP[j*384+c] = 256j+c          (c < 256, h)
                                 = 1024+128j+c-256  (c >= 256, m)
  state tile (t, j): IP[(t*4+j)*128+c] = 256j+128t+c (t<2, h)
                                       = 1024+128j+c (t=2, m)
The state tile (t, j) is st[:, 128t+32j : +16] after transpose, and
weight block r = t*4+j rows follow IP so state/weights stay aligned.

Empirical notes (HW traces):
- PSUM dependency tracking is tile-granular: column-splitting rounds to
  unblock the first cast early does not help (v5 regression), and the
  three transposes + tanh/copy serialize via the shared tp tile.
- unroll=112 and restructured transposes (v6/v7) perturbed the PE clock
  / scheduler into ~183ns round pitch; this arrangement reaches ~163ns.
"""

import numpy as np

import concourse.bass as bass
import concourse.mybir as mybir
import concourse.tile as tile
from concourse import bacc
from concourse.bass import ds, ts
from concourse.bass_utils import run_bass_kernel_spmd
from concourse.masks import make_identity

T, B, UNITS, ORDER = 784, 128, 1024, 512
NCORES = 8
BL = B // NCORES          # 16 sequences per core
SOUT = UNITS + ORDER      # 1536 output features [pre_h; m']
KT = 12                   # state K-tiles of 128
NSTRIP = 4                # PE column groups
NW = SOUT // NSTRIP       # 384 output columns per strip

FP16 = mybir.dt.float16
FP32 = mybir.dt.float32


def _perms():
    OP = np.empty(SOUT, np.int64)
    for j in range(NSTRIP):
        for c in range(NW):
            OP[j * NW + c] = 256 * j + c if c < 256 else 1024 + 128 * j + (c - 256)
    IP = np.empty(SOUT, np.int64)
    for t in range(3):
        for j in range(4):
            for c in range(128):
                IP[(t * 4 + j) * 128 + c] = (
                    256 * j + 128 * t + c if t < 2 else 1024 + 128 * j + c
                )
    return OP, IP


def _build_weights(e_x, e_h, e_m, W_x, W_h, W_m, AT, BT):
    """Host-side fold into M_big (float64), then permute rows/cols."""
    f = np.float64
    e_x, e_h, e_m = e_x.astype(f), e_h.astype(f), e_m.astype(f)
    W_x, W_h, W_m = W_x.astype(f), W_h.astype(f), W_m.astype(f)
    AT, BT = AT.astype(f), BT.astype(f)
    Ad = np.eye(ORDER) + AT
    w = W_m @ BT                                   # (U, 1)
    top = np.concatenate([W_h + w @ e_h, W_m @ Ad + w @ e_m, W_x + w * e_x], axis=1)
    bot = np.concatenate([BT @ e_h, Ad + BT @ e_m, BT * e_x], axis=1)
    M_big = np.concatenate([top, bot], axis=0)     # (1536 out, 1537 in)
    OP, IP = _perms()
    Wstate = M_big[np.ix_(OP, IP)].T               # (in-perm, out-slot)
    xw = M_big[OP, SOUT]                           # x column, out-slot order
    return Wstate.astype(np.float16), xw.reshape(1, SOUT).astype(np.float16)


def _build_nc(t_steps=T, unroll=56):
    assert t_steps % unroll == 0 and unroll % 2 == 0
    iters = t_steps // unroll
    nc = bacc.Bacc("TRN2", target_bir_lowering=False, num_devices=NCORES)

    w_dram = nc.dram_tensor("wts", [KT * 128, SOUT], FP16, kind="ExternalInput")
    xw_dram = nc.dram_tensor("xw", [NSTRIP, NW], FP16, kind="ExternalInput")
    wd_dram = nc.dram_tensor("wd", [UNITS + 1, 10], FP16, kind="ExternalInput")
    # one body of zero padding at the end: body i prefetches slice i+1
    x_dram = nc.dram_tensor(
        "xs", [NSTRIP, (t_steps + unroll) * 128], FP16, kind="ExternalInput"
    )
    out_dram = nc.dram_tensor("out", [BL, 10], FP32, kind="ExternalOutput")

    TANH = mybir.ActivationFunctionType.Tanh

    with tile.TileContext(nc) as tc:
        with (
            tc.tile_pool(name="const", bufs=1) as cpool,
            tc.tile_pool(name="state", bufs=1) as spool,
            tc.tile_pool(name="work", bufs=2) as wpool,
            tc.tile_pool(name="psum", bufs=1, space="PSUM") as ppool,
        ):
            # ---- persistent SBUF ----
            w_sb = cpool.tile([128, KT * SOUT], FP16, tag="w_sb")
            for r in range(KT):
                nc.sync.dma_start(w_sb[:, ts(r, SOUT)], w_dram[ts(r, 128), :])
            xw_sb = cpool.tile([NSTRIP, NW], FP16, tag="xw_sb")
            nc.sync.dma_start(xw_sb[:, :], xw_dram[:, :])
            wd_sb = cpool.tile([128, 8 * 10], FP16, tag="wd_sb")
            for r in range(8):
                nc.sync.dma_start(wd_sb[:, ts(r, 10)], wd_dram[ts(r, 128), :])
            bias_sb = cpool.tile([1, 10], FP16, tag="bias_sb")
            nc.sync.dma_start(bias_sb[:, :], wd_dram[1024:1025, :])
            ones_sb = cpool.tile([1, BL], FP16, tag="ones_sb")
            nc.vector.memset(ones_sb[:, :], 1.0)
            ident = cpool.tile([128, 128], FP16, tag="ident")
            make_identity(nc, ident[:, :])

            stA = spool.tile([128, 3 * 128], FP16, tag="stA")
            stB = spool.tile([128, 3 * 128], FP16, tag="stB")
            sfA = spool.tile([128, NW], FP16, tag="sfA")
            sfB = spool.tile([128, NW], FP16, tag="sfB")
            for t_ in (stA, stB, sfA, sfB):
                nc.vector.memset(t_[:, :], 0.0)
            x_stage = spool.tile([NSTRIP, unroll * 128], FP16, tag="x_stage")
            # prologue: body 0's x slice
            nc.sync.dma_start(x_stage[:, :], x_dram[:, 0 : unroll * 128])

            psA = ppool.tile([128, NW], FP32, tag="psA")
            psB = ppool.tile([128, NW], FP32, tag="psB")
            tpA = ppool.tile([128, 3 * 128], FP16, tag="tpA")
            tpB = ppool.tile([128, 3 * 128], FP16, tag="tpB")

            def xround(ps, x_ap):
                # all four strips' rank-1 x contribution in ONE K=4 matmul
                nc.tensor.matmul(
                    ps[:, :], x_ap, xw_sb[:, :], start=True, stop=False,
                )

            def step(src, dst, ps, ps_next, tp, sf, x_next):
                # next step's x round first: its deps (x_stage, ps_next) are
                # ready early, so the scheduler can slot it into the PE gap
                # while this step waits on the previous tanh
                if x_next is not None:
                    xround(ps_next, x_next)
                # 12 K-rounds; within a round the 4 strip matmuls go to the
                # four PE column groups and stream concurrently.  The last
                # round is split column-wise (cols 128:NW first, 0:128
                # last-but-short).
                for r in range(KT):
                    lhsT = src[:, ds(128 * (r // 4) + 32 * (r % 4), BL)]
                    if r < KT - 1:
                        for j in range(NSTRIP):
                            nc.tensor.matmul(
                                ps[32 * j : 32 * j + BL, :],
                                lhsT,
                                w_sb[:, ds(r * SOUT + j * NW, NW)],
                                start=False,
                                stop=False,
                                tile_position=(0, 32 * j),
                            )
                    else:
                        for j in range(NSTRIP):
                            nc.tensor.matmul(
                                ps[32 * j : 32 * j + BL, 128:NW],
                                lhsT,
                                w_sb[:, ds(r * SOUT + j * NW + 128, NW - 128)],
                                start=False,
                                stop=True,
                                tile_position=(0, 32 * j),
                            )
                        for j in range(NSTRIP):
                            nc.tensor.matmul(
                                ps[32 * j : 32 * j + BL, 0:128],
                                lhsT,
                                w_sb[:, ds(r * SOUT + j * NW, 128)],
                                start=False,
                                stop=True,
                                tile_position=(0, 32 * j),
                            )
                # raw psum -> sbuf fp16 (pre-activation); chunk 0 first so
                # the critical chain to the next step's rounds unblocks asap
                nc.vector.tensor_copy(sf[:, 0:128], ps[:, 0:128])
                nc.vector.tensor_copy(sf[:, 128:NW], ps[:, 128:NW])
                nc.tensor.transpose(tp[:, 0:128], sf[:, 0:128], ident[:, :])
                # finalize state: tanh on h chunks, copy on m chunk
                nc.scalar.activation(dst[:, 0:128], tp[:, 0:128], TANH)
                nc.tensor.transpose(tp[:, 128:256], sf[:, 128:256], ident[:, :])
                nc.scalar.activation(dst[:, 128:256], tp[:, 128:256], TANH)
                nc.tensor.transpose(tp[:, 256:384], sf[:, 256:384], ident[:, :])
                nc.vector.tensor_copy(dst[:, 256:384], tp[:, 256:384])

            with tc.For_i(0, iters, hint_engines=(mybir.EngineType.PE,)) as i:
                xround(psA, x_stage[:, 0:128])
                for u in range(unroll):
                    src, dst = (stA, stB) if u % 2 == 0 else (stB, stA)
                    ps, psn = (psA, psB) if u % 2 == 0 else (psB, psA)
                    tp = tpA if u % 2 == 0 else tpB
                    sf = sfA if u % 2 == 0 else sfB
                    x_next = (
                        x_stage[:, ts(u + 1, 128)] if u < unroll - 1 else None
                    )
                    step(src, dst, ps, psn, tp, sf, x_next)
                # prefetch next body's x while the last steps still run;
                # the WAR on x_stage (last read at step unroll-2) orders it
                nc.sync.dma_start(
                    x_stage[:, :],
                    x_dram[:, ds((i + 1) * (unroll * 128), unroll * 128)],
                )

            # ---- epilogue: logits = h W_d^T + b ; softmax ----
            # final state is in stA (t_steps even); h = chunks t=0,1
            ps_l = ppool.tile([BL, 10], FP32, tag="ps_l")
            for kt in range(8):
                nc.tensor.matmul(
                    ps_l[:, :],
                    stA[:, ds(128 * (kt // 4) + 32 * (kt % 4), BL)],
                    wd_sb[:, ts(kt, 10)],
                    start=(kt == 0),
                    stop=False,
                )
            nc.tensor.matmul(
                ps_l[:, :], ones_sb[:, :], bias_sb[:, :], start=False, stop=True
            )
            sm = wpool.tile([BL, 10], FP32, tag="sm")
            nc.scalar.activation(sm[:, :], ps_l[:, :], mybir.ActivationFunctionType.Exp)
            ssum = wpool.tile([BL, 1], FP32, tag="ssum")
            nc.vector.reduce_sum(ssum[:, :], sm[:, :], axis=mybir.AxisListType.X)
            srec = wpool.tile([BL, 1], FP32, tag="srec")
            nc.vector.reciprocal(srec[:, :], ssum[:, :])
            nc.vector.tensor_scalar_mul(sm[:, :], sm[:, :], srec[:, :])
            nc.sync.dma_start(out_dram[:, :], sm[:, :])

    nc.compile()
    return nc


_NC_CACHE = {}


def _get_nc(t_steps=T, unroll=56):
    key = (t_steps, unroll)
    if key not in _NC_CACHE:
        _NC_CACHE[key] = _build_nc(t_steps, unroll)
    return _NC_CACHE[key]


def kernel(inputs, e_x, e_h, e_m, W_x, W_h, W_m, AT, BT, W_dense, b_dense,
           _t_steps=T, _unroll=56, _trace=False):
    inputs = np.asarray(inputs, np.float32)
    args = [np.asarray(a, np.float32)
            for a in (e_x, e_h, e_m, W_x, W_h, W_m, AT, BT, W_dense, b_dense)]
    e_x, e_h, e_m, W_x, W_h, W_m, AT, BT, W_dense, b_dense = args

    wts, xw = _build_weights(e_x, e_h, e_m, W_x, W_h, W_m, AT, BT)
    xw = np.ascontiguousarray(xw.reshape(NSTRIP, NW))
    _, IP = _perms()
    wd = np.zeros((UNITS + 1, 10), np.float16)
    wd[:UNITS, :] = W_dense.T[IP[:UNITS], :].astype(np.float16)
    wd[UNITS, :] = b_dense.astype(np.float16)

    x = inputs[:_t_steps, :, 0].astype(np.float16)        # (T, B)
    nc = _get_nc(_t_steps, _unroll)
    in_maps = []
    for c in range(NCORES):
        xc = x[:, c * BL:(c + 1) * BL]                    # (T, BL)
        # x-spread for the single K=4 x matmul: row k carries x_t at
        # columns t*128 + 32k + b; one body of zero padding at the end
        xs4 = np.zeros((NSTRIP, _t_steps + _unroll, 128), np.float16)
        for k in range(NSTRIP):
            xs4[k, :_t_steps, 32 * k : 32 * k + BL] = xc
        xs4 = np.ascontiguousarray(
            xs4.reshape(NSTRIP, (_t_steps + _unroll) * 128)
        )
        in_maps.append({"wts": wts, "xw": xw, "wd": wd, "xs": xs4})

    res = run_bass_kernel_spmd(
        nc, in_maps, core_ids=list(range(NCORES)), trace=_trace
    )
    out = np.concatenate([res.results[c]["out"] for c in range(NCORES)], axis=0)
    kernel.last_results = res
    return out.astype(np.float32)
